# revision 1
# baseline (speedup 1.0000x reference)
"""GPT forward (8 layers, C=1024, T=1024, B=2, H=16, V=32000) on 8 trn2 cores.

Sharding: TP4 x DP2. Cores 0-3 handle batch 0, cores 4-7 batch 1.
Within a quad, core j owns heads 4j..4j+3, MLP hidden slice j*1024..,
and vocab slice j*8000.. of the LM head.

Device layout: the residual stream lives in SBUF transposed (xT: [C, T],
channels on partitions). All matmuls contract over the partition dim, so
weights (w[C,F] etc.) are natively the stationary lhsT operand and no
activation transposes are ever needed. LN stats (sums over C) are computed
on the PE with a ones[128,1] stationary vector. Softmax is max-free (logits
are provably tiny) with the denominator fused into the AV matmul via a ones
column appended to V. Matmuls run in bf16 with fp32 PSUM accumulation;
residual/softmax/LN math stays fp32.
"""

import numpy as np
import ml_dtypes

import concourse.bacc as bacc
import concourse.bass as bass
import concourse.tile as tile
import concourse.mybir as mybir
from concourse import bass_utils

f32 = mybir.dt.float32
bf16 = mybir.dt.bfloat16
AF = mybir.ActivationFunctionType
OP = mybir.AluOpType

B, T, C, L, H, F, V = 2, 1024, 1024, 8, 16, 4096, 32000
HD = C // H            # 64
TP = 4                 # tensor-parallel within a quad
HL = H // TP           # 4 local heads
QO = C // TP           # 256 local q/k/v width
FL = F // TP           # 1024 local mlp hidden
VL = V // TP           # 8000 local vocab
NCH = C // 128         # 8 channel chunks
NTC = T // 128         # 8 token chunks
GROUPS = [[0, 1, 2, 3], [4, 5, 6, 7]]
LN_EPS = 1e-5
SCALE = 1.0 / np.sqrt(HD)

_STATE = {}


def _build(collectives=True):
    nc = bacc.Bacc("TRN2", target_bir_lowering=False, debug=False,
                   enable_asserts=False, num_devices=8)

    x0T_d = nc.dram_tensor("x0t", [C, T], f32, kind="ExternalInput").ap()
    wqkv_d = nc.dram_tensor("wqkv", [L, C, 3 * QO], bf16, kind="ExternalInput").ap()
    w1_d = nc.dram_tensor("w1", [L, C, FL], bf16, kind="ExternalInput").ap()
    w2_d = nc.dram_tensor("w2", [L, FL, C], bf16, kind="ExternalInput").ap()
    hw_d = nc.dram_tensor("hw", [C, VL], bf16, kind="ExternalInput").ap()
    # per-partition constant columns (see host packing below)
    bqk_d = nc.dram_tensor("bqk", [128, L * 4], f32, kind="ExternalInput").ap()
    bvb_d = nc.dram_tensor("bvb", [L, 128, QO], f32, kind="ExternalInput").ap()
    b1_d = nc.dram_tensor("b1c", [128, L * 8], f32, kind="ExternalInput").ap()
    b2_d = nc.dram_tensor("b2c", [128, L * 8], f32, kind="ExternalInput").ap()
    ln1w_d = nc.dram_tensor("ln1w", [128, L * 8], f32, kind="ExternalInput").ap()
    ln1b_d = nc.dram_tensor("ln1b", [128, L * 8], f32, kind="ExternalInput").ap()
    ln2w_d = nc.dram_tensor("ln2w", [128, L * 8], f32, kind="ExternalInput").ap()
    ln2b_d = nc.dram_tensor("ln2b", [128, L * 8], f32, kind="ExternalInput").ap()
    lnfw_d = nc.dram_tensor("lnfw", [128, 8], f32, kind="ExternalInput").ap()
    lnfb_d = nc.dram_tensor("lnfb", [128, 8], f32, kind="ExternalInput").ap()
    mask_d = nc.dram_tensor("mask", [128, 128], bf16, kind="ExternalInput").ap()
    out_d = nc.dram_tensor("out", [T, VL], f32, kind="ExternalOutput").ap()

    with tile.TileContext(nc) as tc:
        _prog(nc, tc, x0T_d, wqkv_d, w1_d, w2_d, hw_d, bqk_d, bvb_d, b1_d,
              b2_d, ln1w_d, ln1b_d, ln2w_d, ln2b_d, lnfw_d, lnfb_d, mask_d,
              out_d, collectives)
    nc.compile()
    return nc


def _prog(nc, tc, x0T_d, wqkv_d, w1_d, w2_d, hw_d, bqk_d, bvb_d, b1_d, b2_d,
          ln1w_d, ln1b_d, ln2w_d, ln2b_d, lnfw_d, lnfb_d, mask_d, out_d,
          collectives=True):
    import contextlib
    ctx = contextlib.ExitStack()
    with ctx:
        const = ctx.enter_context(tc.tile_pool(name="const", bufs=1))
        xp = ctx.enter_context(tc.tile_pool(name="xres", bufs=NCH))
        hp = ctx.enter_context(tc.tile_pool(name="hln", bufs=NCH))
        qkp = ctx.enter_context(tc.tile_pool(name="qk", bufs=4))
        vp = ctx.enter_context(tc.tile_pool(name="vsb", bufs=32))
        sbf = ctx.enter_context(tc.tile_pool(name="scrbf", bufs=9))
        s32 = ctx.enter_context(tc.tile_pool(name="scr32", bufs=6))
        bc = ctx.enter_context(tc.tile_pool(name="bcast", bufs=2))
        yp = ctx.enter_context(tc.tile_pool(name="ysb", bufs=2))
        sm = ctx.enter_context(tc.tile_pool(name="small", bufs=4))
        wqp = ctx.enter_context(tc.tile_pool(name="wqkv", bufs=10))
        w1p = ctx.enter_context(tc.tile_pool(name="w1", bufs=9))
        w2p = ctx.enter_context(tc.tile_pool(name="w2", bufs=9))
        bvp = ctx.enter_context(tc.tile_pool(name="bvb", bufs=2))
        hwp = ctx.enter_context(tc.tile_pool(name="hwsb", bufs=16))
        psb = ctx.enter_context(tc.tile_pool(name="psbig", bufs=2, space="PSUM"))
        pss = ctx.enter_context(tc.tile_pool(name="pssm", bufs=2, space="PSUM"))
        dr = ctx.enter_context(tc.tile_pool(name="dram", bufs=2, space="DRAM"))

        ones = const.tile([128, 1], f32)
        nc.vector.memset(ones[:], 1.0)
        eps_t = const.tile([1, 1], f32, tag="eps")
        nc.vector.memset(eps_t[:], LN_EPS)
        mask = const.tile([128, 128], bf16)
        nc.sync.dma_start(mask[:], mask_d[:])
        cols = {}
        for nm, d, w in (("bqk", bqk_d, L * 4), ("b1", b1_d, L * 8),
                         ("b2", b2_d, L * 8), ("l1w", ln1w_d, L * 8),
                         ("l1b", ln1b_d, L * 8), ("l2w", ln2w_d, L * 8),
                         ("l2b", ln2b_d, L * 8), ("lfw", lnfw_d, 8),
                         ("lfb", lnfb_d, 8)):
            t = const.tile([128, w], f32, tag=f"c_{nm}")
            nc.sync.dma_start(t[:], d[:])
            cols[nm] = t

        # residual stream: 8 persistent fp32 tiles [128 ch, 1024 tok]
        xt = []
        for cc in range(NCH):
            t = xp.tile([128, T], f32)
            nc.sync.dma_start(t[:], x0T_d[cc * 128:(cc + 1) * 128, :])
            xt.append(t)

        def layernorm(wcol, bcol, coff):
            """xt -> list of 8 bf16 [128,T] normalized tiles."""
            ssum = pss.tile([1, T], f32, tag="pss")
            sqsum = pss.tile([1, T], f32, tag="pss")
            for cc in range(NCH):
                sq = s32.tile([128, T], f32, tag="s32")
                nc.scalar.activation(sq[:], xt[cc][:], AF.Square)
                for th in range(2):
                    sl = slice(th * 512, (th + 1) * 512)
                    nc.tensor.matmul(ssum[:, sl], ones[:], xt[cc][:, sl],
                                     start=(cc == 0), stop=(cc == NCH - 1))
                    nc.tensor.matmul(sqsum[:, sl], ones[:], sq[:, sl],
                                     start=(cc == 0), stop=(cc == NCH - 1))
            mu = sm.tile([1, T], f32, tag="sm")
            nc.vector.tensor_scalar_mul(mu[:], ssum[:], 1.0 / C)
            var = sm.tile([1, T], f32, tag="sm")
            # var = sqsum/C - mu^2  ->  (sqsum * 1/C) sub mu*mu
            mu2 = sm.tile([1, T], f32, tag="sm")
            nc.vector.tensor_mul(mu2[:], mu[:], mu[:])
            nc.vector.scalar_tensor_tensor(var[:], sqsum[:], 1.0 / C, mu2[:],
                                           op0=OP.mult, op1=OP.subtract)
            std = sm.tile([1, T], f32, tag="sm")
            nc.scalar.activation(std[:], var[:], AF.Sqrt, bias=eps_t[:])
            rstd = sm.tile([1, T], f32, tag="sm")
            nc.vector.reciprocal(rstd[:], std[:])
            nmr = sm.tile([1, T], f32, tag="sm")
            nc.vector.scalar_tensor_tensor(nmr[:], mu[:], -1.0, rstd[:],
                                           op0=OP.mult, op1=OP.mult)
            rstd_b = bc.tile([128, T], f32, tag="bc")
            nc.gpsimd.partition_broadcast(rstd_b[:], rstd[:])
            nmr_b = bc.tile([128, T], f32, tag="bc")
            nc.gpsimd.partition_broadcast(nmr_b[:], nmr[:])
            out = []
            for cc in range(NCH):
                t1 = s32.tile([128, T], f32, tag="s32")
                nc.vector.tensor_mul(t1[:], xt[cc][:], rstd_b[:])
                nc.vector.tensor_add(t1[:], t1[:], nmr_b[:])
                h = hp.tile([128, T], bf16)
                co = coff + cc
                nc.scalar.activation(h[:], t1[:], AF.Identity,
                                     scale=wcol[:, co:co + 1],
                                     bias=bcol[:, co:co + 1])
                out.append(h)
            return out

        for l in range(L):
            wq_t = []
            for cc in range(NCH):
                t = wqp.tile([128, 3 * QO], bf16)
                nc.sync.dma_start(t[:], wqkv_d[l, cc * 128:(cc + 1) * 128, :])
                wq_t.append(t)

            h1 = layernorm(cols["l1w"], cols["l1b"], l * 8)

            # q,k in transposed [qo, T] layout (2 chunks each)
            qk_t = []
            for oc in range(4):
                p = psb.tile([128, T], f32, tag="psb")
                for th in range(2):
                    sl = slice(th * 512, (th + 1) * 512)
                    for cc in range(NCH):
                        nc.tensor.matmul(p[:, sl],
                                         wq_t[cc][:, oc * 128:(oc + 1) * 128],
                                         h1[cc][:, sl],
                                         start=(cc == 0), stop=(cc == NCH - 1))
                dst = qkp.tile([128, T], bf16)
                nc.vector.tensor_scalar_add(dst[:], p[:],
                                            cols["bqk"][:, l * 4 + oc:l * 4 + oc + 1])
                qk_t.append(dst)

            # v in normal [tok, vo] layout, split per head with a ones column
            bvt = bvp.tile([128, QO], f32)
            nc.sync.dma_start(bvt[:], bvb_d[l, :, :])
            v_t = [[None] * HL for _ in range(NTC)]
            for tcc in range(NTC):
                pv = pss.tile([128, QO], f32, tag="pss")
                for cc in range(NCH):
                    nc.tensor.matmul(pv[:], h1[cc][:, tcc * 128:(tcc + 1) * 128],
                                     wq_t[cc][:, 2 * QO:3 * QO],
                                     start=(cc == 0), stop=(cc == NCH - 1))
                for hh in range(HL):
                    vt = vp.tile([128, HD + 1], bf16)
                    nc.vector.memset(vt[:, HD:HD + 1], 1.0)
                    nc.vector.tensor_add(vt[:, 0:HD], pv[:, hh * HD:(hh + 1) * HD],
                                         bvt[:, hh * HD:(hh + 1) * HD])
                    v_t[tcc][hh] = vt

            # attention per local head; y accumulated into 2 fp32 tiles [128, T]
            y_sb = [yp.tile([128, T], f32, tag="y", name=f"ysb{i}") for i in range(2)]
            for hh in range(HL):
                qi, ro = hh // 2, (hh % 2) * 64
                att = []
                for si in range(NTC):
                    pa = psb.tile([128, T], f32, tag="psb")
                    lhs = qk_t[2 + qi][ro:ro + 64, si * 128:(si + 1) * 128]
                    for th in range(si // 4, 2):
                        sl = slice(th * 512, (th + 1) * 512)
                        nc.tensor.matmul(pa[:, sl], lhs,
                                         qk_t[qi][ro:ro + 64, sl],
                                         start=True, stop=True)
                    ab = sbf.tile([128, T], bf16, tag="sbf")
                    sc = si * 128
                    if si % 4:
                        nc.vector.memset(ab[:, (si // 4) * 512:sc], 0.0)
                    nc.scalar.activation(ab[:, sc:T], pa[:, sc:T], AF.Exp,
                                         scale=float(SCALE))
                    nc.vector.tensor_mul(ab[:, sc:sc + 128], ab[:, sc:sc + 128],
                                         mask[:])
                    att.append(ab)
                py = pss.tile([HD + 1, T], f32, tag="pss")
                for th in range(2):
                    last = 3 if th == 0 else 7
                    sl = slice(th * 512, (th + 1) * 512)
                    for si in range(last + 1):
                        nc.tensor.matmul(py[:, sl], v_t[si][hh][:],
                                         att[si][:, sl],
                                         start=(si == 0), stop=(si == last))
                den_r = sm.tile([1, T], f32, tag="sm")
                nc.vector.reciprocal(den_r[:], py[HD:HD + 1, :])
                den_b = bc.tile([64, T], f32, tag="bc")
                nc.gpsimd.partition_broadcast(den_b[:], den_r[:])
                nc.vector.tensor_mul(y_sb[hh // 2][(hh % 2) * 64:(hh % 2) * 64 + 64, :],
                                     py[0:HD, :], den_b[:])

            # AllGather y within quad -> full yT, add to residual
            g_in = dr.tile([QO, T], f32, tag="gin")
            for i in range(2):
                nc.sync.dma_start(g_in[i * 128:(i + 1) * 128, :], y_sb[i][:])
            g_out = dr.tile([C, T], f32, tag="gout")
            if collectives is True:
                nc.gpsimd.collective_compute("AllGather", OP.bypass,
                                             replica_groups=GROUPS,
                                             ins=[g_in.opt()], outs=[g_out.opt()])
            elif collectives == "local":
                for q in range(TP):
                    nc.sync.dma_start(g_out[q * QO:(q + 1) * QO, :], g_in[:])
            for cc in range(NCH):
                yt = s32.tile([128, T], f32, tag="s32")
                nc.sync.dma_start(yt[:], g_out[cc * 128:(cc + 1) * 128, :]
                                  if collectives != "skip"
                                  else g_in[(cc % 2) * 128:(cc % 2) * 128 + 128, :])
                nc.vector.tensor_add(xt[cc][:], xt[cc][:], yt[:])

            # MLP
            w1_t, w2_t = [], []
            for cc in range(NCH):
                t = w1p.tile([128, FL], bf16)
                nc.sync.dma_start(t[:], w1_d[l, cc * 128:(cc + 1) * 128, :])
                w1_t.append(t)
                t = w2p.tile([128, C], bf16)
                nc.sync.dma_start(t[:], w2_d[l, cc * 128:(cc + 1) * 128, :])
                w2_t.append(t)

            h2 = layernorm(cols["l2w"], cols["l2b"], l * 8)
            a_t = []
            for fc in range(NCH):
                pm = psb.tile([128, T], f32, tag="psb")
                for th in range(2):
                    sl = slice(th * 512, (th + 1) * 512)
                    for cc in range(NCH):
                        nc.tensor.matmul(pm[:, sl],
                                         w1_t[cc][:, fc * 128:(fc + 1) * 128],
                                         h2[cc][:, sl],
                                         start=(cc == 0), stop=(cc == NCH - 1))
                ga = sbf.tile([128, T], bf16, tag="sbf")
                nc.scalar.activation(ga[:], pm[:], AF.Gelu,
                                     bias=cols["b1"][:, l * 8 + fc:l * 8 + fc + 1])
                a_t.append(ga)

            r_in = dr.tile([C, T], f32, tag="rin")
            for cc in range(NCH):
                pm2 = psb.tile([128, T], f32, tag="psb")
                for th in range(2):
                    sl = slice(th * 512, (th + 1) * 512)
                    for fc in range(NCH):
                        nc.tensor.matmul(pm2[:, sl],
                                         w2_t[fc][:, cc * 128:(cc + 1) * 128],
                                         a_t[fc][:, sl],
                                         start=(fc == 0), stop=(fc == NCH - 1))
                mo = s32.tile([128, T], f32, tag="s32")
                nc.vector.tensor_copy(mo[:], pm2[:])
                nc.sync.dma_start(r_in[cc * 128:(cc + 1) * 128, :], mo[:])
            r_out = dr.tile([C, T], f32, tag="rout")
            if collectives is True:
                nc.gpsimd.collective_compute("AllReduce", OP.add,
                                             replica_groups=GROUPS,
                                             ins=[r_in.opt()], outs=[r_out.opt()])
            elif collectives == "local":
                nc.sync.dma_start(r_out[:], r_in[:])
            for cc in range(NCH):
                rt = s32.tile([128, T], f32, tag="s32")
                nc.sync.dma_start(rt[:], r_out[cc * 128:(cc + 1) * 128, :]
                                  if collectives != "skip"
                                  else r_in[cc * 128:(cc + 1) * 128, :])
                nc.vector.scalar_tensor_tensor(
                    xt[cc][:], rt[:], cols["b2"][:, l * 8 + cc:l * 8 + cc + 1],
                    xt[cc][:], op0=OP.add, op1=OP.add)

        # final LN + LM head (normal orientation: out[tok, vocab])
        hf = layernorm(cols["lfw"], cols["lfb"], 0)
        NVB = (VL + 511) // 512
        for vb in range(NVB):
            vn = min(512, VL - vb * 512)
            rhs_t = []
            for cc in range(NCH):
                wt = hwp.tile([128, 512], bf16)
                nc.sync.dma_start(wt[:, 0:vn],
                                  hw_d[cc * 128:(cc + 1) * 128,
                                       vb * 512:vb * 512 + vn])
                rhs_t.append(wt)
            for tcc in range(NTC):
                ph = psb.tile([128, 512], f32, tag="psb")
                for cc in range(NCH):
                    nc.tensor.matmul(ph[:, 0:vn],
                                     hf[cc][:, tcc * 128:(tcc + 1) * 128],
                                     rhs_t[cc][:, 0:vn],
                                     start=(cc == 0), stop=(cc == NCH - 1))
                so = s32.tile([128, T], f32, tag="s32")
                if tcc % 2:
                    nc.vector.tensor_copy(so[:, 0:vn], ph[:, 0:vn])
                else:
                    nc.scalar.activation(so[:, 0:vn], ph[:, 0:vn], AF.Copy)
                nc.sync.dma_start(out_d[tcc * 128:(tcc + 1) * 128,
                                        vb * 512:vb * 512 + vn],
                                  so[:, 0:vn])


def _prep_inputs(idx, tok_emb, pos_emb, ln1_w, ln1_b, wq, bq, wk, bk, wv, bv,
                 ln2_w, ln2_b, w1, b1, w2, b2, lnf_w, lnf_b, head_w):
    bf = ml_dtypes.bfloat16

    def cols128(a):  # [L, C] -> [128, L*8] per-partition column packing
        a = np.ascontiguousarray(a, np.float32)
        Lx = a.shape[0]
        return a.reshape(Lx, NCH, 128).transpose(2, 0, 1).reshape(128, Lx * NCH)

    mask = np.zeros((128, 128), np.float32)
    p, t = np.meshgrid(np.arange(128), np.arange(128), indexing="ij")
    mask[p <= t] = 1.0
    in_maps = []
    shard_cache = {}
    x0s = [np.ascontiguousarray(
        (tok_emb[np.asarray(idx[g], np.int64)] + pos_emb[0]).T, np.float32)
        for g in range(B)]
    for c in range(8):
        g, j = c // 4, c % 4
        if j in shard_cache:
            m = dict(shard_cache[j])
            m["x0t"] = x0s[g]
            in_maps.append(m)
            continue
        x0 = tok_emb[np.asarray(idx[g], np.int64)] + pos_emb[0]
        m = {
            "x0t": np.ascontiguousarray(x0.T, np.float32),
            "wqkv": np.ascontiguousarray(np.concatenate(
                [wq[:, :, j * QO:(j + 1) * QO], wk[:, :, j * QO:(j + 1) * QO],
                 wv[:, :, j * QO:(j + 1) * QO]], axis=2)).astype(bf),
            "w1": np.ascontiguousarray(w1[:, :, j * FL:(j + 1) * FL]).astype(bf),
            "w2": np.ascontiguousarray(w2[:, j * FL:(j + 1) * FL, :]).astype(bf),
            "hw": np.ascontiguousarray(head_w[:, j * VL:(j + 1) * VL]).astype(bf),
            "bqk": np.ascontiguousarray(np.stack(
                [bq[:, j * QO:(j + 1) * QO].reshape(L, 2, 128),
                 bk[:, j * QO:(j + 1) * QO].reshape(L, 2, 128)],
                axis=1).reshape(L * 4, 128).T, np.float32),
            "bvb": np.ascontiguousarray(np.broadcast_to(
                bv[:, None, j * QO:(j + 1) * QO], (L, 128, QO)), np.float32),
            "b1c": cols128(b1[:, j * FL:(j + 1) * FL]),
            "b2c": cols128(b2),
            "ln1w": cols128(ln1_w), "ln1b": cols128(ln1_b),
            "ln2w": cols128(ln2_w), "ln2b": cols128(ln2_b),
            "lnfw": cols128(lnf_w[None]), "lnfb": cols128(lnf_b[None]),
            "mask": mask.astype(bf),
        }
        m["x0t"] = x0s[g]
        shard_cache[j] = m
        in_maps.append(m)
    return in_maps


def kernel(**inputs):
    if "nc" not in _STATE:
        _STATE["nc"] = _build()
    nc = _STATE["nc"]
    in_maps = _prep_inputs(**{k: np.asarray(v) for k, v in inputs.items()})
    res = bass_utils.run_bass_kernel_spmd(nc, in_maps, core_ids=list(range(8)))
    outs = res.results
    full = np.empty((B, T, V), np.float32)
    for c in range(8):
        g, j = c // 4, c % 4
        full[g, :, j * VL:(j + 1) * VL] = outs[c]["out"]
    return full

